# revision 7
# baseline (speedup 1.0000x reference)
"""Multi-head self-attention (AttnProcessor) on 8 Trainium2 NeuronCores.

All-bf16 variant (rel err ~4e-3, 5x margin):
  - host pre-casts X^T and Wq/Wk/Wv to bf16: ht DMA is 4 MiB, no
    on-chip conversions; y partials stored bf16 (host upcasts, sums)
  - weights DMA'd once, resident across reps
  - q/k projections, vA, scores, oT all bf16 (vA's moving dim is 64,
    where f32r pays a 4x penalty; bf16 also enables FWL weight loads)
  - oT / denominators / output projection stay f32 for exactness
"""

import numpy as np
import ml_dtypes

S = 4096
D = 512
H = 8
HD = 64
NCORES = 8
NB = S // 128  # 32 s/k blocks of 128
NQ = S // 512  # 8 q chunks of 512
import os as _os

SS = int(_os.environ.get("KERNEL_SS", "2"))  # k-blocks per superstep
MMB = int(_os.environ.get("KERNEL_MMB", "2"))  # mm psum bufs
ESB = int(_os.environ.get("KERNEL_ESB", "3"))  # es sbuf bufs

_CACHE = {}


def _build(reps: int = 1):
    import concourse.mybir as mybir
    from concourse import bacc
    from concourse.tile import TileContext

    f32 = mybir.dt.float32
    f32r = mybir.dt.float32r
    bf16 = mybir.dt.bfloat16
    Exp = mybir.ActivationFunctionType.Exp

    nc = bacc.Bacc("TRN2", target_bir_lowering=False, debug=False, num_devices=NCORES)

    ht = nc.dram_tensor("ht", [D, S], bf16, kind="ExternalInput")
    wq = nc.dram_tensor("wq", [D, HD], bf16, kind="ExternalInput")
    wk = nc.dram_tensor("wk", [D, HD], bf16, kind="ExternalInput")
    wv = nc.dram_tensor("wv", [D, HD], bf16, kind="ExternalInput")
    wo = nc.dram_tensor("wo", [HD, D], f32r, kind="ExternalInput")
    y = nc.dram_tensor("y", [S, D], bf16, kind="ExternalOutput")

    with TileContext(nc) as tc:
        with (
            tc.sbuf_pool(name="sb", bufs=1) as sb,
            tc.sbuf_pool(name="work", bufs=2) as work,
        ):
            wq16 = sb.tile([128, 4 * HD], bf16, name="wq16")
            wk16 = sb.tile([128, 4 * HD], bf16, name="wk16")
            wv16 = sb.tile([128, 4 * HD], bf16, name="wv16")
            wo_sb = sb.tile([HD, D], f32r, name="wo_sb")

            # ---- once: weights (resident across reps) ----
            for i in range(4):
                nc.sync.dma_start(
                    wq16[:, i * HD : (i + 1) * HD], wq[i * 128 : (i + 1) * 128, :]
                )
                nc.sync.dma_start(
                    wk16[:, i * HD : (i + 1) * HD], wk[i * 128 : (i + 1) * 128, :]
                )
                nc.sync.dma_start(
                    wv16[:, i * HD : (i + 1) * HD], wv[i * 128 : (i + 1) * 128, :]
                )
            nc.sync.dma_start(wo_sb[:, :], wo[:, :])

            ones = sb.tile([128, 1], f32, name="ones")
            nc.vector.memset(ones[:, :], 1.0)
            ones16 = sb.tile([128, 1], bf16, name="ones16")
            nc.vector.memset(ones16[:, :], 1.0)
            qT = sb.tile([HD, S], bf16, name="qT")
            kT = sb.tile([HD, S], bf16, name="kT")
            vA = sb.tile([128, NB * 65], bf16, name="vA")
            oT = sb.tile([65, S], f32r, name="oT")
            rc = sb.tile([128, NB], f32, name="rc")  # 1/denominator

            def load_ht(dst):
                # ht in column-major chunks: full 512-col groups land
                # progressively so consumption can chase the load
                for jj in range(2):
                    for i in range(4):
                        nc.sync.dma_start(
                            dst[:, i * S + jj * 2048 : i * S + (jj + 1) * 2048],
                            ht[i * 128 : (i + 1) * 128, jj * 2048 : (jj + 1) * 2048],
                        )

            def new_ht16():
                # double-buffered: rep n+1's DMA (issued mid-rep-n) fills
                # the other buffer while rep n still reads its own
                return work.tile([128, 4 * S], bf16, name="ht16", tag="ht", bufs=2)

            # ---- projections + attention, one PSUM pool ----
            # banks: s=4 (2x[128,1024]) + oT=2 + mm=2 -> 8
            with tc.psum_pool(name="ps", bufs=1) as ps:
              ht16 = new_ht16()
              load_ht(ht16)
              ht_next = None
              for _rep in range(reps):

                  def qt_chunk(j, dst, w16):
                      pqk = ps.tile([HD, 512], f32, name="pqk", tag="mm", bufs=MMB)
                      for i in range(4):
                          nc.tensor.matmul(
                              pqk[:, :],
                              w16[:, i * HD : (i + 1) * HD],
                              ht16[:, i * S + j * 512 : i * S + (j + 1) * 512],
                              start=(i == 0),
                              stop=(i == 3),
                          )
                      nc.vector.tensor_copy(dst[:, j * 512 : (j + 1) * 512], pqk[:, :])

                  def va_block(b):
                      psv = ps.tile([128, HD], f32, name="psv", tag="mm", bufs=MMB)
                      for i in range(4):
                          nc.tensor.matmul(
                              psv[:, :],
                              ht16[:, i * S + b * 128 : i * S + (b + 1) * 128],
                              wv16[:, i * HD : (i + 1) * HD],
                              start=(i == 0),
                              stop=(i == 3),
                          )
                      nc.vector.tensor_copy(vA[:, b * 65 : b * 65 + HD], psv[:, :])
                      nc.vector.tensor_copy(vA[:, b * 65 + HD : b * 65 + 65], ones16[:, :])

                  def proj(q):
                      # output projection + normalization for q's 4 row-blocks;
                      # one aggregated 1 MiB (bf16: 0.5 MiB) store per chunk
                      y_sb = work.tile([128, 4 * D], bf16, name="y_sb", tag="y", bufs=2)
                      for bb in range(4):
                          b = q * 4 + bb
                          py = ps.tile([128, D], f32, name="py", tag="mm", bufs=MMB)
                          nc.tensor.matmul(
                              py[:, :],
                              oT[0:HD, b * 128 : (b + 1) * 128],
                              wo_sb[:, :],
                              start=True,
                              stop=True,
                          )
                          nc.vector.tensor_scalar_mul(
                              y_sb[:, bb * D : (bb + 1) * D], py[:, :], rc[:, b : b + 1]
                          )
                      y_view = y[q * 512 : (q + 1) * 512, :].rearrange(
                          "(b p) d -> p b d", p=128
                      )
                      nc.sync.dma_start(
                          y_view, y_sb[:, :].rearrange("p (b d) -> p b d", b=4)
                      )

                  for j in range(4):
                      qt_chunk(j, kT, wk16)
                  qt_chunk(0, qT, wq16)

                  proj_pending = None
                  for q in range(NQ):
                      qs = slice(q * 512, (q + 1) * 512)
                      poT = ps.tile([65, 512], f32, name="poT", tag="oT", bufs=2)
                      kb0 = 0
                      ss_idx = 0
                      while kb0 < NB:
                          w = min(SS, NB - kb0)
                          if q == 0:
                              for t in range(w):
                                  va_block(kb0 + t)
                          pss = ps.tile(
                              [128, SS * 512], f32, name="pss", tag="s", bufs=2
                          )
                          for t in range(w):
                              kb = kb0 + t
                              nc.tensor.matmul(
                                  pss[:, t * 512 : (t + 1) * 512],
                                  kT[:, kb * 128 : (kb + 1) * 128],
                                  qT[:, qs],
                                  start=True,
                                  stop=True,
                              )
                          es = work.tile(
                              [128, SS * 512], bf16, name="es", tag="es", bufs=ESB
                          )
                          nc.scalar.activation(
                              es[:, : w * 512], pss[:, : w * 512], Exp, scale=0.125
                          )
                          for t in range(w):
                              kb = kb0 + t
                              nc.tensor.matmul(
                                  poT[:, :],
                                  vA[:, kb * 65 : (kb + 1) * 65],
                                  es[:, t * 512 : (t + 1) * 512],
                                  start=(kb == 0),
                                  stop=(kb == NB - 1),
                              )
                          kb0 += w
                          ss_idx += 1
                          if q == 5 and ss_idx == 8 and _rep + 1 < reps:
                              # issue next rep's ht DMA now: it streams into
                              # the other buffer under this rep's tail
                              ht_next = new_ht16()
                              load_ht(ht_next)
                          if q == 0 and ss_idx == 4:
                              # second half of kT (its ht columns have landed by now)
                              for j in range(4, NQ):
                                  qt_chunk(j, kT, wk16)
                          if ss_idx == 3:
                              # runway established: slot in next q's projections
                              # and the q+1 query chunk
                              if q + 1 < NQ:
                                  qt_chunk(q + 1, qT, wq16)
                              if proj_pending is not None:
                                  proj(proj_pending)
                                  proj_pending = None
                      nc.vector.tensor_copy(oT[:, qs], poT[:, :])

                      # denominators: [1,128] rows -> [128,1] columns via a tiny
                      # K=1 PE matmul (dcol[p,0] = oT[64, bs][p] * 1)
                      dcol = ps.tile([128, 4], f32, name="dcol", tag="mm", bufs=MMB)
                      for bb in range(4):
                          b = q * 4 + bb
                          nc.tensor.matmul(
                              dcol[:, bb : bb + 1],
                              oT[64:65, b * 128 : (b + 1) * 128].bitcast(f32),
                              ones[64:65, 0:1],
                              start=True,
                              stop=True,
                          )
                      nc.vector.reciprocal(rc[:, q * 4 : q * 4 + 4], dcol[:, :])
                      proj_pending = q
                  proj(proj_pending)
                  if ht_next is not None:
                      ht16 = ht_next
                      ht_next = None

    nc.compile()
    return nc


def _get_nc(reps: int = 1):
    key = ("nc", reps)
    if key not in _CACHE:
        _CACHE[key] = _build(reps)
    return _CACHE[key]


def _make_in_maps(hidden_states, Wq, Wk, Wv, Wo):
    bf = ml_dtypes.bfloat16
    hT = np.ascontiguousarray(hidden_states.reshape(S, D).T.astype(bf))
    in_maps = []
    for c in range(NCORES):
        cs = slice(c * HD, (c + 1) * HD)
        in_maps.append(
            {
                "ht": hT,
                "wq": np.ascontiguousarray(Wq[:, cs].astype(bf)),
                "wk": np.ascontiguousarray(Wk[:, cs].astype(bf)),
                "wv": np.ascontiguousarray(Wv[:, cs].astype(bf)),
                "wo": np.ascontiguousarray(Wo[cs, :]).astype(np.float32),
            }
        )
    return in_maps


def kernel(hidden_states, Wq, Wk, Wv, Wo, b_out):
    from concourse.bass_utils import run_bass_kernel_spmd

    nc = _get_nc()
    in_maps = _make_in_maps(
        np.asarray(hidden_states, np.float32),
        np.asarray(Wq, np.float32),
        np.asarray(Wk, np.float32),
        np.asarray(Wv, np.float32),
        np.asarray(Wo, np.float32),
    )
    res = run_bass_kernel_spmd(nc, in_maps, list(range(NCORES)))
    acc = np.zeros((S, D), dtype=np.float64)
    for c in range(NCORES):
        acc += res.results[c]["y"].astype(np.float64)
    out = acc.astype(np.float32) + np.asarray(b_out, np.float32)[None, :]
    return out.reshape(1, S, D)
